# revision 9
# baseline (speedup 1.0000x reference)
"""Trainium2 Bass/Tile kernel for an RNN-T Joiner:

    enc_p = encoder_out @ W_enc.T + b_enc          (N,200,512)
    dec_p = decoder_out @ W_dec.T + b_dec          (N,50,512)
    act   = tanh(enc_p[:,:,None,:] + dec_p[:,None,:,:])
    out   = act @ W_out.T + b_out                  (N,200,50,500)

Sharding: data-parallel over N=8 -- core i computes batch element i end to
end; the small weight matrices are replicated to every core.

v3 dataflow (per core), on top of the v2 pipeline:
  - the vocab matmul is a mixed-precision hybrid: j-blocks 0,1 run as ONE
    fp8e4m3 DoubleRow matmul (256-deep contraction pair at 0.5 cycles/row)
    plus a second DoubleRow over the e4m3 *residual* of W_out (W = W8 + Wr8
    captures W to ~2^-10 abs), j-blocks 2,3 stay bf16.  Per 128-cell region:
    250+250+500+500 = 1500 PE cycles vs 2000 all-bf16 (1.33x PE).
  - acts for blocks 0,1 are written by the ACT tanh directly as e4m3 into a
    pair-interleaved [128, 2, CELLS] tile (lhsT layout for DoubleRow);
    blocks 2,3 stay bf16 [128, CELLS].
  - broadcast-adds write bf16 pre-act scratch, split DVE/Pool by a tuned
    pattern (Pool's tensor_tensor is ~2.4x slower than its clock).
  - PSUM drains split DVE/ACT: DVE does tensor_tensor(+b_out); ACT-drained
    doubles get b_out pre-accumulated into PSUM by a 1-partition fp8
    DoubleRow (ones x 8*b_out, 250 cycles) so ACT can drain with a plain
    Identity activation (ACT bias is per-partition and cannot add b_out).
  - output staged and stored as fp16 (same bytes as bf16, ~8x less rounding
    noise against the max-abs-error metric).
"""

import numpy as np
from contextlib import ExitStack

N, T, U = 8, 200, 50
E = J = 512
V = 500
CELLS = T * U
P = 128
KB = J // P  # 4 contraction blocks
DBLK = CELLS // 256  # 39 double blocks
TAIL = CELLS - DBLK * 256  # 16
# prologue chunks get per-chunk tanh (startup latency); main chunks are
# grouped, with ONE tanh instruction per (group, j-block) to amortize the
# ACT engine's ~200ns per-instruction bubble.
PRO_CHUNKS = [(0, 4), (4, 4)]
MAIN_GROUPS = [
    [(8, 8), (16, 16)],
    [(32, 32), (64, 32)],
    [(96, 32), (128, 32)],
    [(160, 32), (192, 8)],
]
GMAX = 3200  # max pre-act scratch columns per (group, j-block)
# packed bf16 input layout: one [512, PACKW] tensor, column offsets:
OFF_ENC, OFF_DEC, OFF_WENC, OFF_WDEC, OFF_WOUT = 0, 200, 256, 768, 1280
PACKW = 1784
SPLIT = 1280  # core operands (enc/dec/Wenc/Wdec) | deferred (Wout bf16 half)

# add engine pattern: addc % ADD_MOD < ADD_DVE -> DVE else Pool
ADD_MOD, ADD_DVE = 8, 3
# drain engine pattern: double % MOD == PHASE -> ACT else DVE
DRAIN_ACT_MOD, DRAIN_ACT_PHASE = 4, 1

_NC_CACHE = {}


def _build_nc(loop_n=1, unroll=1):
    """loop_n > 1 wraps the kernel body in a tc.For_i hardware loop and
    unroll > 1 emits the body that many times inside the loop (test.py's
    slope timing).  Input/projection buffers alternate between two parity
    sets so consecutive emissions pipeline."""
    import concourse.mybir as mybir
    import concourse.tile as tile
    from concourse import bacc

    f32 = mybir.dt.float32
    bf16 = mybir.dt.bfloat16
    f16 = mybir.dt.float16
    fp8 = mybir.dt.float8e4
    ADD = mybir.AluOpType.add
    TANH = mybir.ActivationFunctionType.Tanh
    IDENT = mybir.ActivationFunctionType.Identity
    DR = mybir.MatmulPerfMode.DoubleRow

    nc = bacc.Bacc("TRN2", target_bir_lowering=False, debug=False)

    pack_d = nc.dram_tensor("packed", [E, PACKW], bf16, kind="ExternalInput").ap()
    w8_d = nc.dram_tensor("wout8", [P, 2 * 2 * V], fp8, kind="ExternalInput").ap()
    b8_d = nc.dram_tensor("bias8", [1, 2 * V], fp8, kind="ExternalInput").ap()
    bias_d = nc.dram_tensor("biases_pk", [P, 2 * KB], f32, kind="ExternalInput").ap()
    bout_d = nc.dram_tensor("b_out_row", [1, V], bf16, kind="ExternalInput").ap()
    out_d = nc.dram_tensor("logits", [CELLS, V], f16, kind="ExternalOutput").ap()

    with tile.TileContext(nc) as tc, ExitStack() as ctx:
        const = ctx.enter_context(tc.tile_pool(name="const", bufs=1))
        mm_ps = ctx.enter_context(tc.tile_pool(name="mm_ps", bufs=4, space="PSUM"))
        out_pool = ctx.enter_context(tc.tile_pool(name="outp", bufs=3))
        pre_pool = ctx.enter_context(tc.tile_pool(name="prep", bufs=6))
        if loop_n > 1:
            ctx.enter_context(tc.For_i(0, loop_n, 1))

        # acts shared across reps (block b of rep i+1 only overwrites acts
        # columns after rep i's vocab matmuls for those columns have read
        # them -> fine-grained cross-rep pipelining).
        # blocks 0,1: fp8 pair-interleaved [128, 2, CELLS]; blocks 2,3: bf16.
        acts8 = const.tile([P, 2, CELLS], fp8, name="acts8")
        actsb = [const.tile([P, CELLS], bf16, name=f"actsb{b}") for b in range(2)]
        ptiles = {}  # (name, rep parity) -> tile

        def ctile_for(par):
            def ctile(shape, dtype, name):
                key = (name, par)
                if key not in ptiles:
                    ptiles[key] = const.tile(shape, dtype, name=f"{name}_p{par}")
                return ptiles[key]
            return ctile

        def emit_add(pre, off, t0, L, jb, enc_pT, dec_pT, addc):
            C = L * U
            s = pre[:, off : off + C]
            add_eng = nc.vector if addc % ADD_MOD < ADD_DVE else nc.gpsimd
            add_eng.tensor_tensor(
                out=s.rearrange("p (l u) -> p l u", u=U),
                in0=dec_pT[jb][:, None, :].broadcast_to([P, L, U]),
                in1=enc_pT[jb][:, t0 : t0 + L][:, :, None].broadcast_to([P, L, U]),
                op=ADD,
            )

        def emit_tanh(pre, jb, c0, C):
            if jb < 2:
                nc.scalar.activation(acts8[:, jb, c0 : c0 + C], pre[:, :C], TANH)
            else:
                nc.scalar.activation(actsb[jb - 2][:, c0 : c0 + C], pre[:, :C], TANH)

        def prologue(rep):
            """Loads + projections + b_out broadcast + the first two t-chunks,
            emitted mid-way through the PREVIOUS rep's vocab loop."""
            par = rep % 2
            ctile = ctile_for(par)
            bias_sb = ctile([P, 2 * KB], f32, "bias")
            nc.sync.dma_start(bias_sb[:], bias_d[:, :])
            b_enc_sb = bias_sb[:, 0:KB]
            b_dec_sb = bias_sb[:, KB : 2 * KB]
            big = [ctile([P, PACKW], bf16, f"pack{kb}") for kb in range(KB)]
            for kb in range(KB):
                nc.sync.dma_start(
                    big[kb][:, :SPLIT], pack_d[kb * P : (kb + 1) * P, :SPLIT]
                )
            encT = [b[:, OFF_ENC : OFF_ENC + T] for b in big]
            decT = [b[:, OFF_DEC : OFF_DEC + U] for b in big]
            W_encT = [b[:, OFF_WENC : OFF_WENC + J] for b in big]
            W_decT = [b[:, OFF_WDEC : OFF_WDEC + J] for b in big]
            bout_sb = ctile([1, V], bf16, "bout")
            nc.sync.dma_start(bout_sb[:], bout_d[:, :])
            w8_sb = ctile([P, 2, 2 * V], fp8, "w8")
            nc.sync.dma_start(
                w8_sb[:], w8_d[:, :].rearrange("p (i w) -> p i w", i=2)
            )
            b8_sb = ctile([1, 2, V], fp8, "b8")
            nc.sync.dma_start(b8_sb[:], b8_d[:, :].rearrange("p (i v) -> p i v", i=2))
            # bf16 W_out for blocks 2,3 (rows 256..511 of the pack)
            for kb in (2, 3):
                nc.sync.dma_start(
                    big[kb][:, SPLIT:PACKW], pack_d[kb * P : (kb + 1) * P, SPLIT:PACKW]
                )
            ones1 = ctile([1, P], bf16, "ones")
            ones8 = ctile([1, 2, P], fp8, "ones8")
            if rep < 2:
                nc.gpsimd.memset(ones1[:], 1.0)
                nc.gpsimd.memset(ones8[:], 0.0625)

            if rep == 0:
                # warm the PE clock gate during the input-DMA window
                wps = mm_ps.tile([P, 1024], f32, tag="mm", name=f"warm_ps{rep}")
                for _ in range(12):
                    nc.tensor.matmul(
                        wps[:, :P], lhsT=ones1[:], rhs=ones1[:], start=True, stop=True
                    )
                # prefetch the ACT engine's tanh table while the DMAs run
                warm = ctile([1, 8], f32, "warm")
                nc.gpsimd.memset(warm[:], 0.0)
                nc.scalar.activation(warm[:], warm[:], TANH)

            def project_jb(WT, srcT, b_sb, width, nm, jb):
                pp = mm_ps.tile([P, 1024], f32, tag="mm", name=f"{nm}_ps{jb}_{rep}")[
                    :, :width
                ]
                for kb in range(KB):
                    nc.tensor.matmul(
                        pp[:],
                        lhsT=WT[kb][:, jb * P : (jb + 1) * P],
                        rhs=srcT[kb][:],
                        start=(kb == 0),
                        stop=(kb == KB - 1),
                    )
                o = ctile([P, width], f32, f"{nm}{jb}")
                nc.scalar.activation(o[:], pp[:], IDENT, bias=b_sb[:, jb : jb + 1])
                return o

            enc_pT, dec_pT = [], []
            for jb in range(KB):
                enc_pT.append(project_jb(W_encT, encT, b_enc_sb, T, "encp", jb))
                dec_pT.append(project_jb(W_decT, decT, b_dec_sb, U, "decp", jb))

            bp = mm_ps.tile([P, 1024], f32, tag="mm", name=f"bout_ps{rep}")[:, :V]
            nc.tensor.matmul(bp[:], lhsT=ones1[:], rhs=bout_sb[:], start=True, stop=True)
            bout_rep = ctile([P, V], bf16, "bout_rep")
            nc.vector.tensor_copy(bout_rep[:], bp[:])

            addc = 0
            for t0, L in PRO_CHUNKS:
                for jb in range(KB):
                    pre = pre_pool.tile([P, GMAX], bf16, tag="pre", name=f"prp{addc}")
                    emit_add(pre, 0, t0, L, jb, enc_pT, dec_pT, addc)
                    emit_tanh(pre, jb, t0 * U, L * U)
                    addc += 1
            W_outT = [big[kb][:, OFF_WOUT : OFF_WOUT + V] for kb in (2, 3)]
            W8 = w8_sb[:, :, 0:V]
            Wr8 = w8_sb[:, :, V : 2 * V]
            ops = dict(
                W_outT=W_outT, W8=W8, Wr8=Wr8, ones8=ones8, b8=b8_sb,
                bout_rep=bout_rep,
            )
            return enc_pT, dec_pT, ops, addc

        def main(rep, pro, next_pro):
            """Remaining chunks + the vocab loop."""
            enc_pT, dec_pT, ops, addc = pro
            QD = 4  # double blocks per staging tile / DMA batch
            PRO_AT = 30

            def vocab_region(reg, cells_lo, n_cells, with_bias):
                first = not with_bias
                if with_bias:
                    nc.tensor.matmul(
                        reg,
                        lhsT=ops["ones8"][:, :, :n_cells],
                        rhs=ops["b8"][:],
                        start=True,
                        stop=False,
                        perf_mode=DR,
                    )
                nc.tensor.matmul(
                    reg,
                    lhsT=acts8[:, :, cells_lo : cells_lo + n_cells],
                    rhs=ops["W8"],
                    start=first,
                    stop=False,
                    perf_mode=DR,
                )
                nc.tensor.matmul(
                    reg,
                    lhsT=acts8[:, :, cells_lo : cells_lo + n_cells],
                    rhs=ops["Wr8"],
                    start=False,
                    stop=False,
                    perf_mode=DR,
                )
                for i, b in enumerate((0, 1)):
                    nc.tensor.matmul(
                        reg,
                        lhsT=actsb[b][:, cells_lo : cells_lo + n_cells],
                        rhs=ops["W_outT"][b][:],
                        start=False,
                        stop=(i == 1),
                    )

            pend = []  # drained-but-not-DMAd (q, staging tile, first double)
            flush_plan = [1, 1, 2] if rep == 0 else []  # then QD

            def flush_dma():
                if not pend:
                    return
                nd = len(pend)
                ob = pend[0][1]
                d0 = pend[0][2]
                c0 = d0 * 256
                dst = out_d[c0 : c0 + nd * 256, :].rearrange("(b p) v -> p b v", p=P)
                nc.sync.dma_start(
                    dst, ob[:, : nd * 2 * V].rearrange("p (b v) -> p b v", v=V)
                )
                pend.clear()

            def emit_double(d):
                act_drain = d % DRAIN_ACT_MOD == DRAIN_ACT_PHASE
                ps = mm_ps.tile([P, 1024], f32, tag="mm", name=f"ps{d}_{rep}")
                for r in range(2):
                    vocab_region(
                        ps[:, r * 512 : r * 512 + V], d * 256 + r * P, P, act_drain
                    )
                q = len(pend)
                ob = pend[0][1] if pend else out_pool.tile(
                    [P, QD * 2 * V], f16, tag="ob", name=f"ob{d}_{rep}"
                )
                dst = ob[:, q * 2 * V : (q + 1) * 2 * V].rearrange(
                    "p (b v) -> p b v", v=V
                )
                src = ps.rearrange("p (b q) -> p b q", q=512)[:, :, :V]
                if act_drain:
                    nc.scalar.activation(dst, src, IDENT)
                else:
                    nc.vector.tensor_tensor(
                        out=dst,
                        in0=src,
                        in1=ops["bout_rep"][:, None, :].broadcast_to([P, 2, V]),
                        op=ADD,
                    )
                pend.append((q, ob, d - q))
                target = flush_plan[0] if flush_plan else QD
                if len(pend) >= target:
                    if flush_plan:
                        flush_plan.pop(0)
                    flush_dma()

            db = 0
            # doubles already covered by the prologue chunks
            pro_cov = (PRO_CHUNKS[-1][0] + PRO_CHUNKS[-1][1]) * U
            while db < DBLK and (db + 1) * 256 <= pro_cov:
                emit_double(db)
                db += 1
            for group in MAIN_GROUPS:
                g0 = group[0][0] * U
                pres = [
                    pre_pool.tile([P, GMAX], bf16, tag="pre", name=f"prg{addc}_{jb}")
                    for jb in range(KB)
                ]
                for t0, L in group:
                    for jb in range(KB):
                        emit_add(
                            pres[jb], t0 * U - g0, t0, L, jb, enc_pT, dec_pT, addc
                        )
                        addc += 1
                gend = (group[-1][0] + group[-1][1]) * U
                for jb in range(KB):
                    emit_tanh(pres[jb], jb, g0, gend - g0)
                while db < DBLK and (db + 1) * 256 <= gend:
                    emit_double(db)
                    db += 1
                    if db == PRO_AT and next_pro is not None:
                        next_pro()
            flush_dma()
            if TAIL:
                ps = mm_ps.tile([P, 1024], f32, tag="mm", name=f"ps_tail{rep}")
                vocab_region(ps[:TAIL, :V], DBLK * 256, TAIL, False)
                obt = out_pool.tile(
                    [P, QD * 2 * V], f16, tag="ob", name=f"ob_tail{rep}"
                )
                nc.vector.tensor_tensor(
                    out=obt[:TAIL, :V],
                    in0=ps[:TAIL, :V],
                    in1=ops["bout_rep"][:TAIL, :],
                    op=ADD,
                )
                nc.sync.dma_start(out_d[DBLK * 256 : CELLS, :], obt[:TAIL, :V])

        pros = {0: prologue(0)}

        def make_next(rep):
            if rep + 1 >= unroll:
                return None
            def emit():
                pros[rep + 1] = prologue(rep + 1)
            return emit

        for rep in range(unroll):
            main(rep, pros[rep], make_next(rep))

    nc.compile()
    return nc


def get_nc(loop_n=1, unroll=1):
    key = (loop_n, unroll)
    if key not in _NC_CACHE:
        _NC_CACHE[key] = _build_nc(loop_n, unroll)
    return _NC_CACHE[key]


def make_in_maps(inputs):
    import concourse.mybir as mybir

    bf = mybir.dt.np(mybir.dt.bfloat16)
    e4 = mybir.dt.np(mybir.dt.float8e4)

    enc = np.asarray(inputs["encoder_out"], dtype=np.float32)
    dec = np.asarray(inputs["decoder_out"], dtype=np.float32)
    w_pack = np.zeros((E, PACKW), dtype=np.float32)
    w_pack[:, OFF_WENC : OFF_WENC + J] = np.asarray(inputs["W_enc"]).T
    w_pack[:, OFF_WDEC : OFF_WDEC + J] = np.asarray(inputs["W_dec"]).T
    w_pack[:, OFF_WOUT : OFF_WOUT + V] = np.asarray(inputs["W_out"]).T
    biases = np.empty((P, 2 * KB), dtype=np.float32)
    biases[:, 0:KB] = np.asarray(inputs["b_enc"], dtype=np.float32).reshape(KB, P).T
    biases[:, KB : 2 * KB] = (
        np.asarray(inputs["b_dec"], dtype=np.float32).reshape(KB, P).T
    )
    bout = np.asarray(inputs["b_out"], dtype=np.float32).reshape(1, V)

    # fp8 vocab operands for j-blocks 0,1: W8 = e4m3(W), Wr8 = e4m3(W - W8),
    # pair-interleaved [128, i, v] with i = block index; bias8 = e4m3(8*b_out)
    # per pair slot, consumed via a ones(=1/16) DoubleRow so that the two
    # slots sum to b_out.
    Wo = np.asarray(inputs["W_out"], dtype=np.float32)  # [V, J]
    w8pk = np.zeros((P, 2, 2 * V), dtype=np.float32)
    for i in range(2):
        blk = Wo[:, i * P : (i + 1) * P].T  # [P, V]
        hi = blk.astype(e4).astype(np.float32)
        w8pk[:, i, 0:V] = hi
        w8pk[:, i, V : 2 * V] = blk - hi
    bias8 = np.broadcast_to(8.0 * bout, (2, V)).reshape(1, 2 * V)

    maps = []
    for i in range(N):
        pk = w_pack.copy()
        pk[:, OFF_ENC : OFF_ENC + T] = enc[i].T
        pk[:, OFF_DEC : OFF_DEC + U] = dec[i].T
        maps.append(
            {
                "packed": np.ascontiguousarray(pk.astype(bf)),
                "wout8": np.ascontiguousarray(
                    w8pk.reshape(P, 4 * V).astype(e4)
                ),
                "bias8": bias8.astype(e4),
                "biases_pk": biases,
                "b_out_row": bout.astype(bf),
            }
        )
    return maps


def kernel(**inputs):
    from concourse.bass_utils import run_bass_kernel_spmd

    nc = get_nc()
    in_maps = make_in_maps(inputs)
    res = run_bass_kernel_spmd(nc, in_maps, core_ids=list(range(N)))
    out = np.stack(
        [np.asarray(r["logits"], dtype=np.float32) for r in res.results], axis=0
    )
    return out.reshape(N, T, U, V)


# revision 11
# speedup vs baseline: 1.1042x; 1.1042x over previous
"""Trainium2 Bass/Tile kernel for an RNN-T Joiner:

    enc_p = encoder_out @ W_enc.T + b_enc          (N,200,512)
    dec_p = decoder_out @ W_dec.T + b_dec          (N,50,512)
    act   = tanh(enc_p[:,:,None,:] + dec_p[:,None,:,:])
    out   = act @ W_out.T + b_out                  (N,200,50,500)

Sharding: data-parallel over N=8 -- core i computes batch element i end to
end; the small weight matrices are replicated to every core.

v4 dataflow (per core) = the proven v2 pipeline with a mixed-precision
vocab matmul exploiting the measured fp8 quad-pump:
  - HW measurement: an fp8e4m3 DoubleRow matmul streams ~4 moving rows per
    cycle ONLY when the moving burst is <=512 rows and the access is
    64-aligned (out width 256: ~136 cycles; out width 500: ~500 cycles,
    no better than bf16).  So the fp8 half of the contraction is emitted
    as TWO 256-wide DoubleRow matmuls over a V-padded-to-512 weight tile.
  - j-blocks 0,1 (256 of 512 contraction): acts written by tanh directly
    as e4m3 into a pair-interleaved [128, 2, ACELLS] tile; vocab uses
    W8 = e4m3(W) plus a second DoubleRow pair over the e4m3 residual
    Wr8 = e4m3(W - W8), which restores W to ~2^-10 abs.  j-blocks 2,3
    stay bf16 end to end (v2 scheme: in-place add + tanh).
    Per 128-cell region: 4 x ~136 + 2 x ~430 = ~1400 PE cycles vs ~1700
    all-bf16.  Full-fp8 would halve again but its acts error (~2e-2)
    sits on the correctness gate, so only half the contraction goes fp8
    (measured rel err ~1.6e-2).
  - output staged and stored as fp16 (same bytes as bf16, ~8x less
    rounding noise against the max-abs-error metric).
"""

import numpy as np
from contextlib import ExitStack

N, T, U = 8, 200, 50
E = J = 512
V = 512  # padded vocab width (last 12 weight columns zero)
VO = 500  # true vocab width
CELLS = T * U
ACELLS = 10240  # acts8 row pitch, 64-aligned
P = 128
KB = J // P  # 4 contraction blocks
DBLK = CELLS // 256  # 39 double blocks
TAIL = CELLS - DBLK * 256  # 16
CHUNKS = (
    [(0, 4), (4, 4), (8, 8)]
    + [(t0, 16) for t0 in range(16, 192, 16)]
    + [(192, 8)]
)
# packed input layout: one [512, PACKW] bf16 tensor, column offsets:
OFF_ENC, OFF_DEC, OFF_WENC, OFF_WDEC, OFF_WOUT = 0, 200, 256, 768, 1280
PACKW = 1280 + V
SPLIT = 1280  # core operands (enc/dec/Wenc/Wdec) | deferred (Wout bf16 half)

_NC_CACHE = {}


def _build_nc(loop_n=1, unroll=1):
    """loop_n > 1 wraps the kernel body in a tc.For_i hardware loop and
    unroll > 1 emits the body that many times inside the loop; one dispatch
    executes the kernel loop_n*unroll times (test.py slope timing).
    Input/projection buffers alternate between two parity sets so
    consecutive emissions pipeline."""
    import concourse.mybir as mybir
    import concourse.tile as tile
    from concourse import bacc

    f32 = mybir.dt.float32
    bf16 = mybir.dt.bfloat16
    f16 = mybir.dt.float16
    fp8 = mybir.dt.float8e4
    ADD = mybir.AluOpType.add
    TANH = mybir.ActivationFunctionType.Tanh
    IDENT = mybir.ActivationFunctionType.Identity
    DR = mybir.MatmulPerfMode.DoubleRow

    nc = bacc.Bacc("TRN2", target_bir_lowering=False, debug=False)

    pack_d = nc.dram_tensor("packed", [E, PACKW], bf16, kind="ExternalInput").ap()
    w8_d = nc.dram_tensor("wout8", [P, 2 * 2 * V], fp8, kind="ExternalInput").ap()
    bias_d = nc.dram_tensor("biases_pk", [P, 2 * KB], f32, kind="ExternalInput").ap()
    bout_d = nc.dram_tensor("b_out_row", [1, VO], bf16, kind="ExternalInput").ap()
    out_d = nc.dram_tensor("logits", [CELLS, VO], f16, kind="ExternalOutput").ap()

    with tile.TileContext(nc) as tc, ExitStack() as ctx:
        const = ctx.enter_context(tc.tile_pool(name="const", bufs=1))
        mm_ps = ctx.enter_context(tc.tile_pool(name="mm_ps", bufs=4, space="PSUM"))
        out_pool = ctx.enter_context(tc.tile_pool(name="outp", bufs=3))
        pre_pool = ctx.enter_context(tc.tile_pool(name="prep", bufs=6))
        if loop_n > 1:
            ctx.enter_context(tc.For_i(0, loop_n, 1))

        # acts are shared across reps: block b of rep i+1 only overwrites
        # acts columns after rep i's vocab matmuls for those columns have
        # read them, which gives fine-grained cross-rep pipelining.
        # blocks 0,1: fp8 pair-interleaved; blocks 2,3: bf16 (v2 scheme).
        acts8 = const.tile([P, 2, ACELLS], fp8, name="acts8")
        actsb = [const.tile([P, CELLS], bf16, name=f"actsb{b}") for b in range(2)]
        ptiles = {}  # (name, rep parity) -> tile
        PRO_CHUNKS = CHUNKS[:2]   # emitted in the prologue (covers double 0)
        MAIN_CHUNKS = CHUNKS[2:]

        def ctile_for(par):
            def ctile(shape, dtype, name):
                key = (name, par)
                if key not in ptiles:
                    ptiles[key] = const.tile(shape, dtype, name=f"{name}_p{par}")
                return ptiles[key]
            return ctile

        def emit_chunk(t0, L, jb, enc_pT, dec_pT, addc):
            c0 = t0 * U
            C = L * U
            add_eng = nc.vector if addc % 8 in (0, 2, 5) else nc.gpsimd
            if jb < 2:
                # add -> bf16 scratch, tanh converts into the fp8 pair tile
                pre = pre_pool.tile([P, 800], bf16, tag="pre", name=f"pre{addc}")
                s = pre[:, :C]
            else:
                s = actsb[jb - 2][:, c0 : c0 + C]
            add_eng.tensor_tensor(
                out=s.rearrange("p (l u) -> p l u", u=U),
                in0=dec_pT[jb][:, None, :].broadcast_to([P, L, U]),
                in1=enc_pT[jb][:, t0 : t0 + L][:, :, None].broadcast_to([P, L, U]),
                op=ADD,
            )
            if jb < 2:
                nc.scalar.activation(acts8[:, jb, c0 : c0 + C], s, TANH)
            else:
                nc.scalar.activation(s, s, TANH)

        def prologue(rep):
            """Loads + projections + b_out broadcast + the first two
            t-chunks, emitted mid-way through the PREVIOUS rep's vocab loop
            so this ladder overlaps the previous rep's tail."""
            par = rep % 2
            ctile = ctile_for(par)
            bias_sb = ctile([P, 2 * KB], f32, "bias")
            nc.sync.dma_start(bias_sb[:], bias_d[:, :])
            b_enc_sb = bias_sb[:, 0:KB]
            b_dec_sb = bias_sb[:, KB : 2 * KB]
            big = [ctile([P, PACKW], bf16, f"pack{kb}") for kb in range(KB)]
            for kb in range(KB):
                nc.sync.dma_start(
                    big[kb][:, :SPLIT], pack_d[kb * P : (kb + 1) * P, :SPLIT]
                )
            encT = [b[:, OFF_ENC : OFF_ENC + T] for b in big]
            decT = [b[:, OFF_DEC : OFF_DEC + U] for b in big]
            W_encT = [b[:, OFF_WENC : OFF_WENC + J] for b in big]
            W_decT = [b[:, OFF_WDEC : OFF_WDEC + J] for b in big]
            bout_sb = ctile([1, VO], bf16, "bout")
            nc.sync.dma_start(bout_sb[:], bout_d[:, :])
            w8_sb = ctile([P, 2, 2 * V], fp8, "w8")
            nc.sync.dma_start(w8_sb[:], w8_d[:, :].rearrange("p (i w) -> p i w", i=2))
            # bf16 W_out for blocks 2,3 (rows 256..511 of the pack)
            for kb in (2, 3):
                nc.sync.dma_start(
                    big[kb][:, SPLIT:PACKW], pack_d[kb * P : (kb + 1) * P, SPLIT:PACKW]
                )
            ones1 = ctile([1, P], bf16, "ones")
            if rep < 2:
                nc.gpsimd.memset(ones1[:], 1.0)

            if rep == 0:
                # warm the PE clock gate during the input-DMA window
                wps = mm_ps.tile([P, 1024], f32, tag="mm", name=f"warm_ps{rep}")
                for _ in range(20):
                    nc.tensor.matmul(
                        wps[:, :P], lhsT=ones1[:], rhs=ones1[:], start=True, stop=True
                    )
                # prefetch the ACT engine's tanh table while the DMAs run
                warm = ctile([1, 8], f32, "warm")
                nc.gpsimd.memset(warm[:], 0.0)
                nc.scalar.activation(warm[:], warm[:], TANH)

            def project_jb(WT, srcT, b_sb, width, nm, jb):
                pp = mm_ps.tile([P, 1024], f32, tag="mm", name=f"{nm}_ps{jb}_{rep}")[
                    :, :width
                ]
                for kb in range(KB):
                    nc.tensor.matmul(
                        pp[:],
                        lhsT=WT[kb][:, jb * P : (jb + 1) * P],
                        rhs=srcT[kb][:],
                        start=(kb == 0),
                        stop=(kb == KB - 1),
                    )
                o = ctile([P, width], f32, f"{nm}{jb}")
                nc.scalar.activation(o[:], pp[:], IDENT, bias=b_sb[:, jb : jb + 1])
                return o

            enc_pT, dec_pT = [], []
            for jb in range(KB):
                enc_pT.append(project_jb(W_encT, encT, b_enc_sb, T, "encp", jb))
                dec_pT.append(project_jb(W_decT, decT, b_dec_sb, U, "decp", jb))

            bp = mm_ps.tile([P, 1024], f32, tag="mm", name=f"bout_ps{rep}")[:, :VO]
            nc.tensor.matmul(bp[:], lhsT=ones1[:], rhs=bout_sb[:], start=True, stop=True)
            bout_rep = ctile([P, VO], f32, "bout_rep")
            nc.vector.tensor_copy(bout_rep[:], bp[:])

            addc = 0
            for t0, L in PRO_CHUNKS:
                for jb in range(KB):
                    emit_chunk(t0, L, jb, enc_pT, dec_pT, addc)
                    addc += 1
            W_outT = {kb: big[kb][:, OFF_WOUT : OFF_WOUT + V] for kb in (2, 3)}
            ops = dict(
                W_outT=W_outT,
                W8=w8_sb[:, :, 0:V],
                Wr8=w8_sb[:, :, V : 2 * V],
                bout_rep=bout_rep,
            )
            return enc_pT, dec_pT, ops, addc

        def main(rep, pro, next_pro):
            """Remaining chunks + the vocab loop.  next_pro is called after
            vocab double PRO_AT to emit the NEXT rep's prologue."""
            enc_pT, dec_pT, ops, addc = pro
            QD = 4  # double blocks per staging tile / DMA batch
            PRO_AT = 30

            def vocab_region(reg512, cells_lo, n_cells):
                """reg512: [n_cells, 512] psum view.  One accumulation group
                per 256-wide half (PSUM allows a single pending group per
                bank): fp8 quad-pumped DoubleRow pair (blocks 0,1) + its
                e4m3-residual pair, then the bf16 blocks 2,3."""
                a = acts8[:, :, cells_lo : cells_lo + n_cells]
                for h in (0, 1):
                    sl = slice(h * 256, (h + 1) * 256)
                    reg = reg512[:, sl]
                    for W, st in ((ops["W8"], True), (ops["Wr8"], False)):
                        nc.tensor.matmul(
                            reg,
                            lhsT=a,
                            rhs=W[:, :, sl],
                            start=st,
                            stop=False,
                            perf_mode=DR,
                        )
                    for i, kb in enumerate((2, 3)):
                        nc.tensor.matmul(
                            reg,
                            lhsT=actsb[i][:, cells_lo : cells_lo + n_cells],
                            rhs=ops["W_outT"][kb][:, sl],
                            start=False,
                            stop=(i == 1),
                        )

            pend = []  # drained-but-not-DMAd (q, staging tile, first double)
            flush_plan = [1, 1, 2] if rep == 0 else []  # then QD

            def flush_dma():
                if not pend:
                    return
                nd = len(pend)
                ob = pend[0][1]
                d0 = pend[0][2]
                c0 = d0 * 256
                dst = out_d[c0 : c0 + nd * 256, :].rearrange("(b p) v -> p b v", p=P)
                nc.sync.dma_start(
                    dst, ob[:, : nd * 2 * VO].rearrange("p (b v) -> p b v", v=VO)
                )
                pend.clear()

            def emit_double(d):
                ps = mm_ps.tile([P, 1024], f32, tag="mm", name=f"ps{d}_{rep}")
                for r in range(2):
                    vocab_region(ps[:, r * 512 : (r + 1) * 512], d * 256 + r * P, P)
                q = len(pend)
                ob = pend[0][1] if pend else out_pool.tile(
                    [P, QD * 2 * VO], f16, tag="ob", name=f"ob{d}_{rep}"
                )
                nc.vector.tensor_tensor(
                    out=ob[:, q * 2 * VO : (q + 1) * 2 * VO].rearrange(
                        "p (b v) -> p b v", v=VO
                    ),
                    in0=ps.rearrange("p (b q) -> p b q", q=512)[:, :, :VO],
                    in1=ops["bout_rep"][:, None, :].broadcast_to([P, 2, VO]),
                    op=ADD,
                )
                pend.append((q, ob, d - q))
                target = flush_plan[0] if flush_plan else QD
                if len(pend) >= target:
                    if flush_plan:
                        flush_plan.pop(0)
                    flush_dma()

            db = 0
            for t0, L in MAIN_CHUNKS:
                for jb in range(KB):
                    emit_chunk(t0, L, jb, enc_pT, dec_pT, addc)
                    addc += 1
                covered = (t0 + L) * U
                while db < DBLK and (db + 1) * 256 <= covered:
                    emit_double(db)
                    db += 1
                    if db == PRO_AT and next_pro is not None:
                        next_pro()
            flush_dma()
            if TAIL:
                ps = mm_ps.tile([P, 1024], f32, tag="mm", name=f"ps_tail{rep}")
                vocab_region(ps[:TAIL, :512], DBLK * 256, TAIL)
                obt = out_pool.tile(
                    [P, QD * 2 * VO], f16, tag="ob", name=f"ob_tail{rep}"
                )
                nc.vector.tensor_tensor(
                    out=obt[:TAIL, :VO],
                    in0=ps[:TAIL, :VO],
                    in1=ops["bout_rep"][:TAIL, :],
                    op=ADD,
                )
                nc.sync.dma_start(out_d[DBLK * 256 : CELLS, :], obt[:TAIL, :VO])

        pros = {0: prologue(0)}

        def make_next(rep):
            if rep + 1 >= unroll:
                return None
            def emit():
                pros[rep + 1] = prologue(rep + 1)
            return emit

        for rep in range(unroll):
            main(rep, pros[rep], make_next(rep))

    nc.compile()
    return nc


def get_nc(loop_n=1, unroll=1):
    key = (loop_n, unroll)
    if key not in _NC_CACHE:
        _NC_CACHE[key] = _build_nc(loop_n, unroll)
    return _NC_CACHE[key]


def make_in_maps(inputs):
    import concourse.mybir as mybir

    bf = mybir.dt.np(mybir.dt.bfloat16)
    e4 = mybir.dt.np(mybir.dt.float8e4)

    enc = np.asarray(inputs["encoder_out"], dtype=np.float32)
    dec = np.asarray(inputs["decoder_out"], dtype=np.float32)
    w_pack = np.zeros((E, PACKW), dtype=np.float32)
    w_pack[:, OFF_WENC : OFF_WENC + J] = np.asarray(inputs["W_enc"]).T
    w_pack[:, OFF_WDEC : OFF_WDEC + J] = np.asarray(inputs["W_dec"]).T
    w_pack[:, OFF_WOUT : OFF_WOUT + VO] = np.asarray(inputs["W_out"]).T  # cols +VO..+V stay zero
    biases = np.empty((P, 2 * KB), dtype=np.float32)
    biases[:, 0:KB] = np.asarray(inputs["b_enc"], dtype=np.float32).reshape(KB, P).T
    biases[:, KB : 2 * KB] = (
        np.asarray(inputs["b_dec"], dtype=np.float32).reshape(KB, P).T
    )
    bout = np.asarray(inputs["b_out"], dtype=np.float32).reshape(1, VO)

    # fp8 vocab operands for j-blocks 0,1, V padded to 512:
    # W8 = e4m3(W), Wr8 = e4m3(W - W8), pair-interleaved [128, i, v].
    Wo = np.asarray(inputs["W_out"], dtype=np.float32)  # [VO, J]
    w8pk = np.zeros((P, 2, 2 * V), dtype=np.float32)
    for i in range(2):
        blk = Wo[:, i * P : (i + 1) * P].T  # [P, VO]
        hi = blk.astype(e4).astype(np.float32)
        w8pk[:, i, 0:VO] = hi
        w8pk[:, i, V : V + VO] = blk - hi
    maps = []
    for i in range(N):
        pk = w_pack.copy()
        pk[:, OFF_ENC : OFF_ENC + T] = enc[i].T
        pk[:, OFF_DEC : OFF_DEC + U] = dec[i].T
        maps.append(
            {
                "packed": np.ascontiguousarray(pk.astype(bf)),
                "wout8": np.ascontiguousarray(w8pk.reshape(P, 4 * V).astype(e4)),
                "biases_pk": biases,
                "b_out_row": bout.astype(bf),
            }
        )
    return maps


def kernel(**inputs):
    from concourse.bass_utils import run_bass_kernel_spmd

    nc = get_nc()
    in_maps = make_in_maps(inputs)
    res = run_bass_kernel_spmd(nc, in_maps, core_ids=list(range(N)))
    out = np.stack(
        [np.asarray(r["logits"], dtype=np.float32) for r in res.results], axis=0
    )
    return out.reshape(N, T, U, VO)


# revision 15
# speedup vs baseline: 1.3107x; 1.1870x over previous
"""Trainium2 Bass/Tile kernel for an RNN-T Joiner:

    enc_p = encoder_out @ W_enc.T + b_enc          (N,200,512)
    dec_p = decoder_out @ W_dec.T + b_dec          (N,50,512)
    act   = tanh(enc_p[:,:,None,:] + dec_p[:,None,:,:])
    out   = act @ W_out.T + b_out                  (N,200,50,500)

Sharding: data-parallel over N=8 -- core i computes batch element i end to
end; the small weight matrices are replicated to every core.

v2 dataflow (per core):
  - the host pre-transposes every contraction operand (encT/decT/WencT/
    WdecT/WoutT, contraction dim leading) and converts them to bf16, so
    SBUF loads are straight DMAs and every matmul runs at 1 cycle/row,
  - projections: psum[j,t] = sum_e WencT.T @ encT (bf16), drained by the
    ACT engine with the per-partition bias folded in -> enc_pT/dec_pT f32,
  - acts: DVE broadcast-add (enc_pT[t] + dec_pT[u]) writes bf16 directly
    into a full-size [128, 10000] act tile per j-block; ACT applies tanh
    in place.  t-chunks grow [4,8,16,32,64,64,12] so the vocab matmul can
    start ~2us in,
  - vocab: per 256-cell double block, one 2-bank PSUM tile [128, 1024]
    (regions bank-aligned at 0/512) holding two accumulation groups of 4
    bf16 matmuls each,
  - drain: one DVE tensor_tensor per double block adds the pre-broadcast
    b_out and moves PSUM->SBUF (GPSIMD cannot read PSUM on TRN2); to
    compensate, 3 of 4 broadcast-adds run on GPSIMD,
  - output: 1MB DMA batches (4 double blocks per SBUF staging tile).
"""

import numpy as np
from contextlib import ExitStack

N, T, U = 8, 200, 50
E = J = 512
V = 500
VO = 500  # true vocab width (= V; kept for test.py compatibility)
CELLS = T * U
P = 128
KB = J // P  # 4 contraction blocks
DBLK = CELLS // 256  # 39 double blocks
TAIL = CELLS - DBLK * 256  # 16
CHUNKS = (
    [(0, 4), (4, 4), (8, 8)]
    + [(t0, 16) for t0 in range(16, 192, 16)]
    + [(192, 8)]
)
# packed input layout: one [512, PACKW] bf16 tensor, column offsets:
OFF_ENC, OFF_DEC, OFF_WENC, OFF_WDEC, OFF_WOUT = 0, 200, 256, 768, 1280
PACKW = 1784

_NC_CACHE = {}
SPLIT = 1280  # core operands (enc/dec/Wenc/Wdec) | deferred (Wout)


def _build_nc(loop_n=1, unroll=1):
    """loop_n > 1 wraps the kernel body in a tc.For_i hardware loop and
    unroll > 1 emits the body that many times inside the loop.  Every
    emission performs the complete kernel (input DMAs included) with the
    same DRAM in/out, so one dispatch executes the kernel loop_n*unroll
    times; test.py uses this to measure per-execution device time by slope
    with no host/dispatch overhead in the delta.  Input/projection buffers
    alternate between two parity sets so consecutive emissions pipeline:
    rep i+1's loads and projections overlap rep i's vocab tail."""
    import concourse.mybir as mybir
    import concourse.tile as tile
    from concourse import bacc

    f32 = mybir.dt.float32
    bf16 = mybir.dt.bfloat16
    ADD = mybir.AluOpType.add
    TANH = mybir.ActivationFunctionType.Tanh
    IDENT = mybir.ActivationFunctionType.Identity

    nc = bacc.Bacc("TRN2", target_bir_lowering=False, debug=False)

    pack_d = nc.dram_tensor("packed", [E, PACKW], bf16, kind="ExternalInput").ap()
    bias_d = nc.dram_tensor("biases_pk", [P, 2 * KB], f32, kind="ExternalInput").ap()
    bout_d = nc.dram_tensor("b_out_row", [1, V], bf16, kind="ExternalInput").ap()
    out_d = nc.dram_tensor("logits", [CELLS, V], bf16, kind="ExternalOutput").ap()

    with tile.TileContext(nc) as tc, ExitStack() as ctx:
        const = ctx.enter_context(tc.tile_pool(name="const", bufs=1))
        mm_ps = ctx.enter_context(tc.tile_pool(name="mm_ps", bufs=4, space="PSUM"))
        out_pool = ctx.enter_context(tc.tile_pool(name="outp", bufs=3))
        if loop_n > 1:
            ctx.enter_context(tc.For_i(0, loop_n, 1))

        # acts are shared across reps: block b of rep i+1 only overwrites
        # acts columns after rep i's vocab matmuls for those columns have
        # read them, which gives fine-grained cross-rep pipelining
        acts = [const.tile([P, CELLS], bf16, name=f"acts{jb}") for jb in range(KB)]
        ptiles = {}  # (name, rep parity) -> tile
        PRO_CHUNKS = CHUNKS[:2]   # emitted in the prologue (covers double 0)
        MAIN_CHUNKS = CHUNKS[2:]

        def ctile_for(par):
            def ctile(shape, dtype, name):
                key = (name, par)
                if key not in ptiles:
                    ptiles[key] = const.tile(shape, dtype, name=f"{name}_p{par}")
                return ptiles[key]
            return ctile

        def emit_chunk(t0, L, jb, enc_pT, dec_pT, addc):
            c0 = t0 * U
            C = L * U
            s = acts[jb][:, c0 : c0 + C]
            add_eng = nc.vector if addc % 8 in (0, 2, 5) else nc.gpsimd
            add_eng.tensor_tensor(
                out=s.rearrange("p (l u) -> p l u", u=U),
                in0=dec_pT[jb][:, None, :].broadcast_to([P, L, U]),
                in1=enc_pT[jb][:, t0 : t0 + L][:, :, None].broadcast_to([P, L, U]),
                op=ADD,
            )
            nc.scalar.activation(s, s, TANH)

        def prologue(rep):
            """Loads + projections + b_out broadcast + the first two
            t-chunks.  Emitted mid-way through the PREVIOUS rep's vocab
            loop so this ladder (which serializes through the ACT engine)
            overlaps the previous rep's tail instead of stalling the PE at
            the rep boundary."""
            par = rep % 2
            ctile = ctile_for(par)
            bias_sb = ctile([P, 2 * KB], f32, "bias")
            nc.sync.dma_start(bias_sb[:], bias_d[:, :])
            b_enc_sb = bias_sb[:, 0:KB]
            b_dec_sb = bias_sb[:, KB : 2 * KB]
            big = [ctile([P, PACKW], bf16, f"pack{kb}") for kb in range(KB)]
            for kb in range(KB):
                nc.sync.dma_start(
                    big[kb][:, :SPLIT], pack_d[kb * P : (kb + 1) * P, :SPLIT]
                )
            encT = [b[:, OFF_ENC : OFF_ENC + T] for b in big]
            decT = [b[:, OFF_DEC : OFF_DEC + U] for b in big]
            W_encT = [b[:, OFF_WENC : OFF_WENC + J] for b in big]
            W_decT = [b[:, OFF_WDEC : OFF_WDEC + J] for b in big]
            bout_sb = ctile([1, V], bf16, "bout")
            nc.sync.dma_start(bout_sb[:], bout_d[:, :])
            for kb in range(KB):
                nc.sync.dma_start(
                    big[kb][:, SPLIT:PACKW], pack_d[kb * P : (kb + 1) * P, SPLIT:PACKW]
                )
            ones1 = ctile([1, P], bf16, "ones")
            if rep < 2:
                nc.gpsimd.memset(ones1[:], 1.0)

            if rep == 0:
                # warm the PE clock gate during the input-DMA window
                wps = mm_ps.tile([P, 1024], f32, tag="mm", name=f"warm_ps{rep}")
                for _ in range(20):
                    nc.tensor.matmul(
                        wps[:, :P], lhsT=ones1[:], rhs=ones1[:], start=True, stop=True
                    )
                # prefetch the ACT engine's tanh table while the DMAs run
                warm = ctile([1, 8], f32, "warm")
                nc.gpsimd.memset(warm[:], 0.0)
                nc.scalar.activation(warm[:], warm[:], TANH)

            def project_jb(WT, srcT, b_sb, width, nm, jb):
                pp = mm_ps.tile([P, 1024], f32, tag="mm", name=f"{nm}_ps{jb}_{rep}")[
                    :, :width
                ]
                for kb in range(KB):
                    nc.tensor.matmul(
                        pp[:],
                        lhsT=WT[kb][:, jb * P : (jb + 1) * P],
                        rhs=srcT[kb][:],
                        start=(kb == 0),
                        stop=(kb == KB - 1),
                    )
                o = ctile([P, width], f32, f"{nm}{jb}")
                nc.scalar.activation(o[:], pp[:], IDENT, bias=b_sb[:, jb : jb + 1])
                return o

            enc_pT, dec_pT = [], []
            for jb in range(KB):
                enc_pT.append(project_jb(W_encT, encT, b_enc_sb, T, "encp", jb))
                dec_pT.append(project_jb(W_decT, decT, b_dec_sb, U, "decp", jb))

            bp = mm_ps.tile([P, 1024], f32, tag="mm", name=f"bout_ps{rep}")[:, :V]
            nc.tensor.matmul(bp[:], lhsT=ones1[:], rhs=bout_sb[:], start=True, stop=True)
            bout_rep = ctile([P, V], f32, "bout_rep")
            nc.vector.tensor_copy(bout_rep[:], bp[:])

            addc = 0
            for t0, L in PRO_CHUNKS:
                for jb in range(KB):
                    emit_chunk(t0, L, jb, enc_pT, dec_pT, addc)
                    addc += 1
            W_outT = [b[:, OFF_WOUT : OFF_WOUT + V] for b in big]
            return enc_pT, dec_pT, bout_rep, W_outT, addc, ones1, bout_sb

        def main(rep, pro, next_pro):
            """Remaining chunks + the vocab loop.  next_pro is called after
            vocab double PRO_AT to emit the NEXT rep's prologue."""
            enc_pT, dec_pT, bout_rep, W_outT, addc, ones1, bout_sb = pro
            QD = 4  # double blocks per staging tile / DMA batch
            PRO_AT = 30

            def vocab_region(reg, cells_lo, n_cells, with_bias=False):
                if with_bias:
                    # pre-accumulate b_out so the ACT engine can drain this
                    # region with a plain Identity (its per-partition bias
                    # cannot add a per-vocab vector)
                    nc.tensor.matmul(
                        reg, lhsT=ones1[:, :n_cells], rhs=bout_sb[:],
                        start=True, stop=False,
                    )
                for jb in range(KB):
                    nc.tensor.matmul(
                        reg,
                        lhsT=acts[jb][:, cells_lo : cells_lo + n_cells],
                        rhs=W_outT[jb][:],
                        start=(jb == 0 and not with_bias),
                        stop=(jb == KB - 1),
                    )

            pend = []  # drained-but-not-DMAd (q, staging tile, first double)
            flush_plan = [1, 1, 2] if rep == 0 else []  # then QD

            def flush_dma():
                if not pend:
                    return
                nd = len(pend)
                ob = pend[0][1]
                d0 = pend[0][2]
                c0 = d0 * 256
                dst = out_d[c0 : c0 + nd * 256, :].rearrange("(b p) v -> p b v", p=P)
                nc.sync.dma_start(
                    dst, ob[:, : nd * 2 * V].rearrange("p (b v) -> p b v", v=V)
                )
                pend.clear()

            def emit_double(d):
                # every 5th double drains on the ACT engine (b_out arrives
                # via the PSUM prefill) to offload the DVE, the baseline's
                # binding engine (drains 53us + adds 19us vs ACT ~51us)
                act_drain = d % 5 == 2
                ps = mm_ps.tile([P, 1024], f32, tag="mm", name=f"ps{d}_{rep}")
                for r in range(2):
                    vocab_region(
                        ps[:, r * 512 : r * 512 + V], d * 256 + r * P, P,
                        with_bias=act_drain,
                    )
                q = len(pend)
                ob = pend[0][1] if pend else out_pool.tile(
                    [P, QD * 2 * V], bf16, tag="ob", name=f"ob{d}_{rep}"
                )
                dst = ob[:, q * 2 * V : (q + 1) * 2 * V].rearrange(
                    "p (b v) -> p b v", v=V
                )
                src = ps.rearrange("p (b q) -> p b q", q=512)[:, :, :V]
                if act_drain:
                    nc.scalar.activation(dst, src, IDENT)
                else:
                    nc.vector.tensor_tensor(
                        out=dst,
                        in0=src,
                        in1=bout_rep[:, None, :].broadcast_to([P, 2, V]),
                        op=ADD,
                    )
                pend.append((q, ob, d - q))
                target = flush_plan[0] if flush_plan else QD
                if len(pend) >= target:
                    if flush_plan:
                        flush_plan.pop(0)
                    flush_dma()

            db = 0
            for t0, L in MAIN_CHUNKS:
                for jb in range(KB):
                    emit_chunk(t0, L, jb, enc_pT, dec_pT, addc)
                    addc += 1
                covered = (t0 + L) * U
                while db < DBLK and (db + 1) * 256 <= covered:
                    emit_double(db)
                    db += 1
                    if db == PRO_AT and next_pro is not None:
                        next_pro()
            flush_dma()
            if TAIL:
                ps = mm_ps.tile([P, 1024], f32, tag="mm", name=f"ps_tail{rep}")
                vocab_region(ps[:TAIL, :V], DBLK * 256, TAIL)
                obt = out_pool.tile(
                    [P, QD * 2 * V], bf16, tag="ob", name=f"ob_tail{rep}"
                )
                nc.vector.tensor_tensor(
                    out=obt[:TAIL, :V],
                    in0=ps[:TAIL, :V],
                    in1=bout_rep[:TAIL, :],
                    op=ADD,
                )
                nc.sync.dma_start(out_d[DBLK * 256 : CELLS, :], obt[:TAIL, :V])

        pros = {0: prologue(0)}

        def make_next(rep):
            if rep + 1 >= unroll:
                return None
            def emit():
                pros[rep + 1] = prologue(rep + 1)
            return emit

        for rep in range(unroll):
            main(rep, pros[rep], make_next(rep))

    nc.compile()
    return nc


def get_nc(loop_n=1, unroll=1):
    key = (loop_n, unroll)
    if key not in _NC_CACHE:
        _NC_CACHE[key] = _build_nc(loop_n, unroll)
    return _NC_CACHE[key]


def make_in_maps(inputs):
    import concourse.mybir as mybir

    bf = mybir.dt.np(mybir.dt.bfloat16)

    enc = np.asarray(inputs["encoder_out"], dtype=np.float32)
    dec = np.asarray(inputs["decoder_out"], dtype=np.float32)
    w_pack = np.zeros((E, PACKW), dtype=np.float32)
    w_pack[:, OFF_WENC : OFF_WENC + J] = np.asarray(inputs["W_enc"]).T
    w_pack[:, OFF_WDEC : OFF_WDEC + J] = np.asarray(inputs["W_dec"]).T
    w_pack[:, OFF_WOUT : OFF_WOUT + V] = np.asarray(inputs["W_out"]).T
    biases = np.empty((P, 2 * KB), dtype=np.float32)
    biases[:, 0:KB] = np.asarray(inputs["b_enc"], dtype=np.float32).reshape(KB, P).T
    biases[:, KB : 2 * KB] = (
        np.asarray(inputs["b_dec"], dtype=np.float32).reshape(KB, P).T
    )
    bout = np.asarray(inputs["b_out"], dtype=np.float32).reshape(1, V).astype(bf)
    maps = []
    for i in range(N):
        pk = w_pack.copy()
        pk[:, OFF_ENC : OFF_ENC + T] = enc[i].T
        pk[:, OFF_DEC : OFF_DEC + U] = dec[i].T
        maps.append(
            {
                "packed": np.ascontiguousarray(pk.astype(bf)),
                "biases_pk": biases,
                "b_out_row": bout,
            }
        )
    return maps


def kernel(**inputs):
    from concourse.bass_utils import run_bass_kernel_spmd

    nc = get_nc()
    in_maps = make_in_maps(inputs)
    res = run_bass_kernel_spmd(nc, in_maps, core_ids=list(range(N)))
    out = np.stack(
        [np.asarray(r["logits"], dtype=np.float32) for r in res.results], axis=0
    )
    return out.reshape(N, T, U, V)
